# revision 1
# baseline (speedup 1.0000x reference)
"""Trainium2 Bass kernel for nn_PartialConvLayer (partial conv 3x3 + mask
update + BatchNorm(batch stats) + ReLU), data-parallel over batch on 8 cores.

Math (per image):
  update = conv(mask, ones(Cin,3,3)), pad 1          # integer in {0..576}
  u      = clip(update, 0, 1)                        # exactly binary
  mr     = 576 / (update + 1e-6) * u
  conv   = conv(x*mask, W), pad 1                    # no bias
  out    = conv * mr + b * u
         = (conv + (b/576) (x) v) * mr,  v = u*(update+1e-6)   [u^2 == u]
  BN over (N,H,W) batch stats (all-reduced across cores), then ReLU.
Returns (out, broadcast(update_clipped)).
"""
import os
import numpy as np
from contextlib import ExitStack

import concourse.bass as bass
import concourse.tile as tile
from concourse import mybir, bacc
from concourse import library_config
from concourse.bass_utils import run_bass_kernel_spmd

F32 = mybir.dt.float32
F32R = mybir.dt.float32r
ALU = mybir.AluOpType
ACTF = mybir.ActivationFunctionType

CIN = 64
COUT = 128
W_ = 256          # image width (fixed: 512-px chunks = 2 rows)
KS = 3
EPS_MASK = 1e-6
EPS_BN = 1e-5
SLIDE = float(CIN * KS * KS)   # 576


def build_nc(n_cores=8, H=256, B=8):
    """SPMD program for one core holding one [CIN, H, W_] image."""
    HB = H // 2                      # rows per band
    nblk = HB // B                   # blocks
    CH_PIX = 512                     # pixels per chunk (2 rows)
    nchunk = (H * W_) // CH_PIX      # chunks per core
    npair = B // 2 + 1               # s row-pairs per block (B+2 rows)
    TOT = float(n_cores * H * W_)    # BN count

    nc = bacc.Bacc(None, num_devices=n_cores)

    X = nc.dram_tensor("x", [CIN, H, W_], F32, kind="ExternalInput")
    M = nc.dram_tensor("mask", [CIN, H, W_], F32R, kind="ExternalInput")
    WT = nc.dram_tensor("wt", [CIN, KS * KS * COUT], F32, kind="ExternalInput")
    BP = nc.dram_tensor("bp", [1, COUT], F32, kind="ExternalInput")   # b/576
    ONES2 = nc.dram_tensor("ones2", [128, 2], F32R, kind="ExternalInput")
    T3 = nc.dram_tensor("t3", [2 * (B + 2), 2 * B], F32R, kind="ExternalInput")
    GAM = nc.dram_tensor("gam", [COUT, 1], F32, kind="ExternalInput")
    BET = nc.dram_tensor("bet", [COUT, 1], F32, kind="ExternalInput")

    OUT = nc.dram_tensor("out", [COUT, H * W_], F32, kind="ExternalOutput")
    UPD = nc.dram_tensor("upd", [H, W_], F32, kind="ExternalOutput")

    prebn = nc.dram_tensor("prebn", [COUT, H * W_], F32)
    s_dram = nc.dram_tensor("sdram", [nblk, 2 * (B + 2) * W_], F32R)
    v_dram = nc.dram_tensor("vdram", [nblk, 2 * B * W_], F32R)
    mru_dram = nc.dram_tensor("mrudram", [nblk, 2 * B * W_], F32)
    cc_in = nc.dram_tensor("ccin", [COUT, 2], F32)
    cc_out = nc.dram_tensor("ccout", [COUT, 2], F32,
                            addr_space="Shared" if n_cores > 4 else "Local")

    with tile.TileContext(nc) as tc, ExitStack() as ctx:
        nc.gpsimd.load_library(library_config.mlp)

        const = ctx.enter_context(tc.tile_pool(name="const", bufs=1))
        io = ctx.enter_context(tc.tile_pool(name="io", bufs=2))
        sblk = ctx.enter_context(tc.tile_pool(name="sblk", bufs=2))
        updp = ctx.enter_context(tc.tile_pool(name="updp", bufs=2))
        chkp = ctx.enter_context(tc.tile_pool(name="chkp", bufs=3))
        p2p = ctx.enter_context(tc.tile_pool(name="p2p", bufs=3))
        psc = ctx.enter_context(tc.tile_pool(name="psc", bufs=3, space="PSUM"))
        pss = ctx.enter_context(tc.tile_pool(name="pss", bufs=2, space="PSUM"))
        psu = ctx.enter_context(tc.tile_pool(name="psu", bufs=2, space="PSUM"))

        # ---- constants ----
        wt_f = const.tile([128, KS * KS * COUT], F32)
        nc.sync.dma_start(wt_f[0:64, :], WT[:])
        nc.sync.dma_start(wt_f[64:128, :], WT[:])
        wt_r = const.tile([128, KS * KS * COUT], F32R)
        nc.vector.tensor_copy(wt_r[:], wt_f[:])
        bp_f = const.tile([1, COUT], F32)
        nc.sync.dma_start(bp_f[:], BP[:])
        bp_r = const.tile([1, COUT], F32R)
        nc.vector.tensor_copy(bp_r[:], bp_f[:])
        ones2_t = const.tile([128, 2], F32R)
        nc.sync.dma_start(ones2_t[:], ONES2[:])
        t3_t = const.tile([2 * (B + 2), 2 * B], F32R)
        nc.sync.dma_start(t3_t[:], T3[:])
        gam_t = const.tile([COUT, 1], F32)
        nc.sync.dma_start(gam_t[:], GAM[:])
        bet_t = const.tile([COUT, 1], F32)
        nc.sync.dma_start(bet_t[:], BET[:])
        eps_t = const.tile([COUT, 1], F32)
        nc.vector.memset(eps_t[:], EPS_BN)
        sum_slots = const.tile([COUT, nchunk], F32)
        sq_slots = const.tile([COUT, nchunk], F32)
        # two persistent padded xm buffers; guard cols zeroed once and
        # never written again (production writes cols 1..256 only)
        xm_tiles = []
        for i in range(2):
            t = const.tile([128, (B + 2) * 258], F32R, tag=f"xm{i}")
            nc.vector.memset(t[:].bitcast(F32), 0.0)
            xm_tiles.append(t)

        ci_global = 0
        for k in range(nblk):
            r0 = k * B
            nrows = B + 2
            # ---- load x, mask band-pair tiles (with halo rows) ----
            x_t = io.tile([128, nrows * W_], F32, tag="x_t")
            m_t = io.tile([128, nrows * W_], F32R, tag="m_t")
            first, last = (k == 0), (k == nblk - 1)
            for tens, tl in ((X, x_t), (M, m_t)):
                if not first and not last:
                    src = bass.AP(tensor=tens, offset=(r0 - 1) * W_,
                                  ap=[[HB * W_, 2], [H * W_, CIN],
                                      [W_, nrows], [1, W_]])
                    nc.sync.dma_start(tl[:, :], src)
                else:
                    # band 0 rows r0-1..r0+B ; band 1 rows r0+HB-1..r0+HB+B
                    b0_lo = max(r0 - 1, 0)
                    b0_n = (r0 + B + 1) - b0_lo
                    b1_hi = min(r0 + HB + B + 1, H)
                    b1_n = b1_hi - (r0 + HB - 1)
                    nc.sync.dma_start(
                        tl[0:64, (b0_lo - (r0 - 1)) * W_:
                                 (b0_lo - (r0 - 1)) * W_ + b0_n * W_],
                        bass.AP(tensor=tens, offset=b0_lo * W_,
                                ap=[[H * W_, CIN], [W_, b0_n], [1, W_]]))
                    nc.sync.dma_start(
                        tl[64:128, 0:b1_n * W_],
                        bass.AP(tensor=tens, offset=(r0 + HB - 1) * W_,
                                ap=[[H * W_, CIN], [W_, b1_n], [1, W_]]))
                    if first:
                        nc.vector.memset(tl[0:64, 0:W_].bitcast(F32), 0.0)
                    if last:
                        nc.vector.memset(tl[64:128, (nrows - 1) * W_:nrows * W_].bitcast(F32), 0.0)

            # ---- xm = x*mask into padded fp32r tile ----
            xm = xm_tiles[k % 2]
            xm3 = xm[:, :].rearrange("p (r c) -> p r c", c=258)
            nc.vector.tensor_tensor(
                xm3[:, :, 1:257],
                x_t[:, :].rearrange("p (r c) -> p r c", c=W_),
                m_t[:, :].rearrange("p (r c) -> p r c", c=W_),
                op=ALU.mult)

            # ---- s = cin-sum of mask, per row-pair, both bands ----
            for p in range(npair):
                ps_s = pss.tile([2, 512], F32, tag="ps_s")
                rhs = m_t[:, :].rearrange("p (r c) -> p r c", c=W_)[:, 2 * p:2 * p + 2, :]
                nc.tensor.matmul(ps_s[:], ones2_t[:], rhs, start=True, stop=True)
                s_pair = sblk.tile([2, 512], F32R, tag="s_pair")
                nc.vector.tensor_copy(s_pair[:], ps_s[:])
                nc.sync.dma_start(
                    bass.AP(tensor=s_dram, offset=k * (2 * (B + 2) * W_) + 2 * p * W_,
                            ap=[[(B + 2) * W_, 2], [1, 512]]),
                    s_pair[:])

            # ---- s_rows <- s_dram ; banded vertical sum ----
            s_rows = sblk.tile([2 * (B + 2), 258], F32R, tag="s_rows")
            nc.vector.memset(s_rows[:, 0:1].bitcast(F32), 0.0)
            nc.vector.memset(s_rows[:, 257:258].bitcast(F32), 0.0)
            nc.sync.dma_start(
                s_rows[:, 1:257],
                bass.AP(tensor=s_dram, offset=k * (2 * (B + 2) * W_),
                        ap=[[W_, 2 * (B + 2)], [1, W_]]))
            ps_u = psu.tile([2 * B, 258], F32, tag="ps_u")
            nc.tensor.matmul(ps_u[:], t3_t[:], s_rows[:, :], start=True, stop=True)
            u_sb = updp.tile([2 * B, 258], F32, tag="u_sb")
            nc.scalar.copy(u_sb[:], ps_u[:])

            # ---- horizontal sum + update math  [2B, 256] ----
            vh = updp.tile([2 * B, W_], F32, tag="vh")
            nc.vector.tensor_add(vh[:], u_sb[:, 0:256], u_sb[:, 1:257])
            nc.vector.tensor_add(vh[:], vh[:], u_sb[:, 2:258])
            u_clip = updp.tile([2 * B, W_], F32, tag="u_clip")
            nc.vector.tensor_scalar_min(u_clip[:], vh[:], 1.0)
            upde = updp.tile([2 * B, W_], F32, tag="upde")
            nc.vector.tensor_scalar_add(upde[:], vh[:], EPS_MASK)
            rec = updp.tile([2 * B, W_], F32, tag="rec")
            nc.vector.reciprocal(rec[:], upde[:])
            mru_rows = updp.tile([2 * B, W_], F32, tag="mru_rows")
            nc.vector.scalar_tensor_tensor(
                out=mru_rows[:], in0=rec[:], scalar=SLIDE, in1=u_clip[:],
                op0=ALU.mult, op1=ALU.mult)
            v_rows = updp.tile([2 * B, W_], F32R, tag="v_rows")
            nc.vector.scalar_tensor_tensor(
                out=v_rows[:], in0=upde[:], scalar=1.0, in1=u_clip[:],
                op0=ALU.mult, op1=ALU.mult)

            # update output + strips via DRAM bounce
            nc.sync.dma_start(
                bass.AP(tensor=UPD, offset=r0 * W_,
                        ap=[[HB * W_, 2], [W_, B], [1, W_]]),
                u_clip[:])
            nc.sync.dma_start(
                bass.AP(tensor=v_dram, offset=k * (2 * B * W_),
                        ap=[[1, 2 * B * W_]]),
                v_rows[:])
            nc.sync.dma_start(
                bass.AP(tensor=mru_dram, offset=k * (2 * B * W_),
                        ap=[[1, 2 * B * W_]]),
                mru_rows[:])

            # ---- conv chunks: per band b, row-pair j ----
            for b in range(2):
                for j in range(0, B, 2):
                    v_strip = chkp.tile([1, 512], F32R, tag="v_strip")
                    nc.sync.dma_start(
                        v_strip[:],
                        bass.AP(tensor=v_dram,
                                offset=k * (2 * B * W_) + (b * B + j) * W_,
                                ap=[[1, 512]]))
                    mru_strip = chkp.tile([1, 512], F32, tag="mru_strip")
                    nc.sync.dma_start(
                        mru_strip[:],
                        bass.AP(tensor=mru_dram,
                                offset=k * (2 * B * W_) + (b * B + j) * W_,
                                ap=[[1, 512]]))
                    mru_bc = chkp.tile([128, 512], F32, tag="mru_bc")
                    nc.gpsimd.partition_broadcast(mru_bc[:], mru_strip[0:1, :])

                    ps_c = psc.tile([COUT, 512], F32, tag="ps_c")
                    for t in range(KS * KS):
                        ky, kx = divmod(t, KS)
                        rhs = xm3[64 * b:64 * b + 64,
                                  j + ky:j + ky + 2, kx:kx + 256]
                        lhsT = wt_r[64 * b:64 * b + 64, t * COUT:(t + 1) * COUT]
                        nc.tensor.matmul(ps_c[:], lhsT, rhs,
                                         start=(t == 0), stop=False)
                    nc.tensor.matmul(ps_c[:], bp_r[:], v_strip[:],
                                     start=False, stop=True)

                    out_sb = chkp.tile([COUT, 512], F32, tag="out_sb")
                    ci = ci_global
                    nc.vector.scalar_tensor_tensor(
                        out=out_sb[:], in0=ps_c[:], scalar=0.0, in1=mru_bc[:],
                        op0=ALU.add, op1=ALU.mult,
                        accum_out=sum_slots[:, ci:ci + 1])
                    sq_scr = chkp.tile([COUT, 512], F32, tag="sq_scr")
                    nc.scalar.activation(
                        sq_scr[:], out_sb[:], ACTF.Square,
                        accum_out=sq_slots[:, ci:ci + 1])
                    row = b * HB + r0 + j
                    nc.sync.dma_start(prebn[:, row * W_:row * W_ + 512], out_sb[:])
                    ci_global += 1

        assert ci_global == nchunk

        # ---- BN stats: reduce, all-reduce, affine coeffs ----
        cc_sb = const.tile([COUT, 2], F32)
        nc.vector.tensor_reduce(cc_sb[:, 0:1], sum_slots[:], axis=mybir.AxisListType.X,
                                op=ALU.add)
        nc.vector.tensor_reduce(cc_sb[:, 1:2], sq_slots[:], axis=mybir.AxisListType.X,
                                op=ALU.add)
        nc.sync.dma_start(cc_in[:], cc_sb[:])
        nc.gpsimd.collective_compute(
            "AllReduce", ALU.add,
            replica_groups=[list(range(n_cores))],
            ins=[cc_in.ap().opt()], outs=[cc_out.ap().opt()])
        st_sb = const.tile([COUT, 2], F32)
        nc.sync.dma_start(st_sb[:], cc_out[:])
        mean_t = const.tile([COUT, 1], F32)
        nc.vector.tensor_scalar_mul(mean_t[:], st_sb[:, 0:1], 1.0 / TOT)
        e2_t = const.tile([COUT, 1], F32)
        nc.vector.tensor_scalar_mul(e2_t[:], st_sb[:, 1:2], 1.0 / TOT)
        msq_t = const.tile([COUT, 1], F32)
        nc.vector.tensor_mul(msq_t[:], mean_t[:], mean_t[:])
        var_t = const.tile([COUT, 1], F32)
        nc.vector.tensor_sub(var_t[:], e2_t[:], msq_t[:])
        std_t = const.tile([COUT, 1], F32)
        nc.scalar.activation(std_t[:], var_t[:], ACTF.Sqrt, bias=eps_t[:])
        rstd_t = const.tile([COUT, 1], F32)
        nc.vector.reciprocal(rstd_t[:], std_t[:])
        scale_t = const.tile([COUT, 1], F32)
        nc.vector.tensor_mul(scale_t[:], gam_t[:], rstd_t[:])
        tmp_t = const.tile([COUT, 1], F32)
        nc.vector.tensor_mul(tmp_t[:], mean_t[:], scale_t[:])
        bias_t = const.tile([COUT, 1], F32)
        nc.vector.tensor_sub(bias_t[:], bet_t[:], tmp_t[:])

        # ---- pass 2: out = relu(scale*prebn + bias) ----
        P2 = 2048
        for i in range(0, H * W_, P2):
            pb_t = p2p.tile([COUT, P2], F32, tag="pb_t")
            nc.sync.dma_start(pb_t[:], prebn[:, i:i + P2])
            o_t = p2p.tile([COUT, P2], F32, tag="o_t")
            nc.scalar.activation(o_t[:], pb_t[:], ACTF.Relu,
                                 bias=bias_t[:], scale=scale_t[:])
            nc.sync.dma_start(OUT[:, i:i + P2], o_t[:])

    return nc


def make_host_inputs(x_i, mask_i, W, b, gamma, beta, B=8):
    """Per-core in_map for one image shard (host-side constant prep)."""
    WT = np.ascontiguousarray(
        W.transpose(1, 2, 3, 0).reshape(CIN, KS * KS * COUT)).astype(np.float32)
    BP = (b / SLIDE).reshape(1, COUT).astype(np.float32)
    ones2 = np.zeros((128, 2), np.float32)
    ones2[0:64, 0] = 1.0
    ones2[64:128, 1] = 1.0
    T3 = np.zeros((2 * (B + 2), 2 * B), np.float32)
    for band in range(2):
        for jj in range(B):
            for d in range(3):
                T3[band * (B + 2) + jj + d, band * B + jj] = 1.0
    return {
        "x": np.ascontiguousarray(x_i, dtype=np.float32),
        "mask": np.ascontiguousarray(mask_i, dtype=np.float32),
        "wt": WT,
        "bp": BP,
        "ones2": ones2,
        "t3": T3,
        "gam": gamma.reshape(COUT, 1).astype(np.float32),
        "bet": beta.reshape(COUT, 1).astype(np.float32),
    }


_NC_CACHE = {}


def kernel(x, mask, W, b, gamma, beta):
    x = np.asarray(x)
    mask = np.asarray(mask)
    W = np.asarray(W)
    b = np.asarray(b)
    gamma = np.asarray(gamma)
    beta = np.asarray(beta)
    N, _, H, _ = x.shape
    n_cores = N
    key = (n_cores, H)
    if key not in _NC_CACHE:
        nc = build_nc(n_cores=n_cores, H=H)
        nc.finalize()
        _NC_CACHE[key] = nc
    nc = _NC_CACHE[key]

    in_maps = [make_host_inputs(x[i], mask[i], W, b, gamma, beta)
               for i in range(n_cores)]
    res = run_bass_kernel_spmd(nc, in_maps, core_ids=list(range(n_cores)),
                               trace=bool(os.environ.get("KERNEL_TRACE")))
    out = np.stack([res.results[i]["out"].reshape(COUT, H, W_)
                    for i in range(n_cores)])
    upd = np.stack([res.results[i]["upd"] for i in range(n_cores)])
    update_full = np.broadcast_to(upd[:, None, :, :], (N, COUT, H, W_))
    kernel.last_result = res
    return out, update_full



# revision 4
# speedup vs baseline: 2.0851x; 2.0851x over previous
"""Trainium2 Bass kernel for nn_PartialConvLayer (partial conv 3x3 + mask
update + BatchNorm(batch stats) + ReLU), data-parallel over batch on 8 cores.

Math (per image):
  update = conv(mask, ones(Cin,3,3)), pad 1          # integer in {0..576}
  u      = clip(update, 0, 1)                        # exactly binary
  mr     = 576 / (update + 1e-6) * u
  conv   = conv(x*mask, W), pad 1                    # no bias
  out    = conv * mr + b * u
         = (conv + (b/576) (x) v) * mr,  v = u*(update+1e-6)   [u^2 == u]
  BN over (N,H,W) batch stats (all-reduced across cores), then ReLU.
Returns (out, broadcast(update_clipped)).

v2 design notes (perf):
  - bf16 activations/weights/outputs (tolerance 2e-2; bf16 error ~0.5%).
  - Input loads split into 8-channel-octet DMA instructions across both
    HWDGE queue families so descriptors execute on many rings in parallel
    (v1 crawled at ~57 GB/s on 2 rings, stalling the PE ~590us and keeping
    it clock-gated at 1.2 GHz).
  - Pre-BN activations kept resident in SBUF as bf16 (128 KB/partition),
    killing the 67 MB DRAM bounce of v1's pass 2.
  - band0 (rows 0..63) / band1 (partitions 64..127) conv matmuls issued
    back-to-back so they run concurrently on different PE row groups.
  - v/mru strip relayout via small SBUF->SBUF DMAs instead of DRAM bounce.
"""
import os
import numpy as np
from contextlib import ExitStack

import ml_dtypes

import concourse.bass as bass
import concourse.tile as tile
from concourse import mybir, bacc
from concourse import library_config
from concourse.bass_utils import run_bass_kernel_spmd

F32 = mybir.dt.float32
BF16 = mybir.dt.bfloat16
ALU = mybir.AluOpType
ACTF = mybir.ActivationFunctionType

CIN = 64
COUT = 128
W_ = 256          # image width
KS = 3
EPS_MASK = 1e-6
EPS_BN = 1e-5
SLIDE = float(CIN * KS * KS)   # 576
NPBF = ml_dtypes.bfloat16


def build_nc(n_cores=8, H=256, B=8):
    """SPMD program for one core holding one [CIN, H, W_] image."""
    HB = H // 2                      # rows per band
    nblk = HB // B                   # blocks
    nrows = B + 2                    # rows per band tile (with halo)
    npair = nrows // 2               # row-pairs for the s matmuls
    nchunk = (H * W_) // 512         # 512-px chunks per core
    TOT = float(n_cores * H * W_)    # BN count
    HW = H * W_

    nc = bacc.Bacc(None, num_devices=n_cores)

    X = nc.dram_tensor("x", [CIN, H, W_], BF16, kind="ExternalInput")
    M = nc.dram_tensor("mask", [CIN, H, W_], BF16, kind="ExternalInput")
    WT = nc.dram_tensor("wt", [128, KS * KS * COUT], BF16, kind="ExternalInput")
    BP2 = nc.dram_tensor("bp2", [128, COUT], BF16, kind="ExternalInput")
    ONES2 = nc.dram_tensor("ones2", [128, 2], BF16, kind="ExternalInput")
    T3 = nc.dram_tensor("t3", [2 * nrows, 2 * B], BF16, kind="ExternalInput")
    GAM = nc.dram_tensor("gam", [COUT, 1], F32, kind="ExternalInput")
    BET = nc.dram_tensor("bet", [COUT, 1], F32, kind="ExternalInput")

    OUT = nc.dram_tensor("out", [COUT, HW], BF16, kind="ExternalOutput")
    UPD = nc.dram_tensor("upd", [H, W_], F32, kind="ExternalOutput")

    cc_in = nc.dram_tensor("ccin", [COUT, 2], F32)
    cc_out = nc.dram_tensor("ccout", [COUT, 2], F32,
                            addr_space="Shared" if n_cores > 4 else "Local")

    with tile.TileContext(nc) as tc, ExitStack() as ctx:
        nc.gpsimd.load_library(library_config.mlp)

        const = ctx.enter_context(tc.tile_pool(name="const", bufs=1))
        io = ctx.enter_context(tc.tile_pool(name="io", bufs=2))
        sblk = ctx.enter_context(tc.tile_pool(name="sblk", bufs=2))
        updp = ctx.enter_context(tc.tile_pool(name="updp", bufs=2))
        strp = ctx.enter_context(tc.tile_pool(name="strp", bufs=3))
        sqp = ctx.enter_context(tc.tile_pool(name="sqp", bufs=2))
        p2p = ctx.enter_context(tc.tile_pool(name="p2p", bufs=3))
        psc = ctx.enter_context(tc.tile_pool(name="psc", bufs=2, space="PSUM"))
        pss = ctx.enter_context(tc.tile_pool(name="pss", bufs=2, space="PSUM"))
        psu = ctx.enter_context(tc.tile_pool(name="psu", bufs=2, space="PSUM"))

        # ---- constants ----
        wt_t = const.tile([128, KS * KS * COUT], BF16)
        nc.sync.dma_start(wt_t[:], WT[:])
        bp_t = const.tile([128, COUT], BF16)
        nc.sync.dma_start(bp_t[:], BP2[:])
        ones2_t = const.tile([128, 2], BF16)
        nc.sync.dma_start(ones2_t[:], ONES2[:])
        t3_t = const.tile([2 * nrows, 2 * B], BF16)
        nc.sync.dma_start(t3_t[:], T3[:])
        gam_t = const.tile([COUT, 1], F32)
        nc.sync.dma_start(gam_t[:], GAM[:])
        bet_t = const.tile([COUT, 1], F32)
        nc.sync.dma_start(bet_t[:], BET[:])
        eps_t = const.tile([COUT, 1], F32)
        nc.vector.memset(eps_t[:], EPS_BN)
        sum_slots = const.tile([COUT, nchunk], F32)
        sq_slots = const.tile([COUT, nchunk], F32)
        # pre-BN activations, SBUF-resident for the whole kernel (bf16)
        prebn = const.tile([COUT, HW], BF16)
        # two persistent padded xm buffers; guard cols 0/257 zeroed once
        xm_tiles = []
        for i in range(2):
            t = const.tile([128, nrows * 258], BF16, tag=f"xm{i}")
            nc.vector.memset(t[:], 0.0)
            xm_tiles.append(t)

        ci_global = 0
        for k in range(nblk):
            r0 = k * B
            first, last = (k == 0), (k == nblk - 1)
            # ---- load x, mask band tiles (halo rows), 8-ch octets ----
            x_t = io.tile([128, nrows * W_], BF16, tag="x_t")
            m_t = io.tile([128, nrows * W_], BF16, tag="m_t")
            for tens, tl, eng in ((X, x_t, nc.sync), (M, m_t, nc.scalar)):
                for b in range(2):
                    rlo = b * HB + r0 - 1          # first halo row
                    rhi = rlo + nrows              # one past last halo row
                    lo = max(rlo, 0)
                    hi = min(rhi, H)
                    c0 = (lo - rlo) * W_           # dest col offset
                    nrw = hi - lo
                    for o in range(8):             # channel octets
                        p0 = b * 64 + 8 * o
                        eng.dma_start(
                            tl[p0:p0 + 8, c0:c0 + nrw * W_],
                            bass.AP(tensor=tens, offset=8 * o * HW + lo * W_,
                                    ap=[[HW, 8], [1, nrw * W_]]))
                    if lo > rlo:    # zero top halo row (block 0, band 0)
                        nc.vector.memset(tl[b * 64:b * 64 + 64, 0:W_], 0.0)
                    if hi < rhi:    # zero bottom halo row (last block, band 1)
                        nc.vector.memset(
                            tl[b * 64:b * 64 + 64, (nrows - 1) * W_:nrows * W_], 0.0)

            x3 = x_t[:, :].rearrange("p (r c) -> p r c", c=W_)
            m3 = m_t[:, :].rearrange("p (r c) -> p r c", c=W_)

            # ---- xm = x*mask into padded bf16 tile ----
            xm = xm_tiles[k % 2]
            xm3 = xm[:, :].rearrange("p (r c) -> p r c", c=258)
            nc.vector.tensor_tensor(xm3[:, :, 1:257], x3, m3, op=ALU.mult)

            # ---- s = cin-sum of mask per row, both bands ----
            s_rows = sblk.tile([2 * nrows, 258], BF16, tag="s_rows")
            nc.vector.memset(s_rows[:, 0:1], 0.0)
            nc.vector.memset(s_rows[:, 257:258], 0.0)
            for p in range(npair):
                ps_s = pss.tile([2, 512], F32, tag="ps_s")
                nc.tensor.matmul(ps_s[:], ones2_t[:], m3[:, 2 * p:2 * p + 2, :],
                                 start=True, stop=True)
                s_pair = sblk.tile([2, 512], BF16, tag="s_pair")
                nc.vector.tensor_copy(s_pair[:], ps_s[:])
                # [2,512] -> rows {2p,2p+1} band0 / {nrows+2p,...} band1
                nc.scalar.dma_start(s_rows[2 * p:2 * p + 2, 1:257],
                                    s_pair[0:1, :])
                nc.scalar.dma_start(
                    s_rows[nrows + 2 * p:nrows + 2 * p + 2, 1:257],
                    s_pair[1:2, :])

            # ---- banded vertical sum via T3 matmul ----
            ps_u = psu.tile([2 * B, 258], F32, tag="ps_u")
            nc.tensor.matmul(ps_u[:], t3_t[:], s_rows[:, :], start=True,
                             stop=True)
            u_sb = updp.tile([2 * B, 258], F32, tag="u_sb")
            nc.scalar.copy(u_sb[:], ps_u[:])

            # ---- horizontal sum + update math  [2B, 256] ----
            vh = updp.tile([2 * B, W_], F32, tag="vh")
            nc.vector.tensor_add(vh[:], u_sb[:, 0:256], u_sb[:, 1:257])
            nc.vector.tensor_add(vh[:], vh[:], u_sb[:, 2:258])
            u_clip = updp.tile([2 * B, W_], F32, tag="u_clip")
            nc.vector.tensor_scalar_min(u_clip[:], vh[:], 1.0)
            upde = updp.tile([2 * B, W_], F32, tag="upde")
            nc.vector.tensor_scalar_add(upde[:], vh[:], EPS_MASK)
            rec = updp.tile([2 * B, W_], F32, tag="rec")
            nc.vector.reciprocal(rec[:], upde[:])
            mru_rows = updp.tile([2 * B, W_], BF16, tag="mru_rows")
            nc.vector.scalar_tensor_tensor(
                out=mru_rows[:], in0=rec[:], scalar=SLIDE, in1=u_clip[:],
                op0=ALU.mult, op1=ALU.mult)
            v_rows = updp.tile([2 * B, W_], BF16, tag="v_rows")
            nc.vector.scalar_tensor_tensor(
                out=v_rows[:], in0=upde[:], scalar=1.0, in1=u_clip[:],
                op0=ALU.mult, op1=ALU.mult)

            nc.scalar.dma_start(
                bass.AP(tensor=UPD, offset=r0 * W_,
                        ap=[[HB * W_, 2], [W_, B], [1, W_]]),
                u_clip[:])

            # ---- conv chunks: per j, band0+band1 paired on row groups ----
            for j in range(0, B, 2):
                vst = strp.tile([128, 512], BF16, tag="vst")
                nc.sync.dma_start(vst[0:1, :], v_rows[j:j + 2, :])
                nc.sync.dma_start(vst[64:65, :], v_rows[B + j:B + j + 2, :])
                mst = strp.tile([1, 1024], BF16, tag="mst")
                nc.sync.dma_start(mst[0:1, 0:512], mru_rows[j:j + 2, :])
                nc.sync.dma_start(mst[0:1, 512:1024],
                                  mru_rows[B + j:B + j + 2, :])
                mru_bc = strp.tile([128, 1024], BF16, tag="mru_bc")
                nc.gpsimd.partition_broadcast(mru_bc[:], mst[0:1, :])

                ps_c0 = psc.tile([COUT, 512], F32, tag="ps_c0")
                ps_c1 = psc.tile([COUT, 512], F32, tag="ps_c1")
                for t in range(KS * KS):
                    ky, kx = divmod(t, KS)
                    lhs_lo = wt_t[0:64, t * COUT:(t + 1) * COUT]
                    lhs_hi = wt_t[64:128, t * COUT:(t + 1) * COUT]
                    nc.tensor.matmul(
                        ps_c0[:], lhs_lo,
                        xm3[0:64, j + ky:j + ky + 2, kx:kx + 256],
                        start=(t == 0), stop=False)
                    nc.tensor.matmul(
                        ps_c1[:], lhs_hi,
                        xm3[64:128, j + ky:j + ky + 2, kx:kx + 256],
                        start=(t == 0), stop=False)
                nc.tensor.matmul(ps_c0[:], bp_t[0:1, :], vst[0:1, :],
                                 start=False, stop=True)
                nc.tensor.matmul(ps_c1[:], bp_t[64:65, :], vst[64:65, :],
                                 start=False, stop=True)

                for b, ps_c in ((0, ps_c0), (1, ps_c1)):
                    off = (b * HB + r0 + j) * W_
                    ci = ci_global + b
                    pslice = prebn[:, off:off + 512]
                    nc.vector.scalar_tensor_tensor(
                        out=pslice, in0=ps_c[:], scalar=0.0,
                        in1=mru_bc[:, 512 * b:512 * b + 512],
                        op0=ALU.add, op1=ALU.mult,
                        accum_out=sum_slots[:, ci:ci + 1])
                    sq_scr = sqp.tile([COUT, 512], BF16, tag="sq_scr")
                    nc.scalar.activation(
                        sq_scr[:], pslice, ACTF.Square,
                        accum_out=sq_slots[:, ci:ci + 1])
                ci_global += 2

        assert ci_global == nchunk

        # ---- BN stats: reduce, all-reduce, affine coeffs ----
        cc_sb = const.tile([COUT, 2], F32)
        nc.vector.tensor_reduce(cc_sb[:, 0:1], sum_slots[:],
                                axis=mybir.AxisListType.X, op=ALU.add)
        nc.vector.tensor_reduce(cc_sb[:, 1:2], sq_slots[:],
                                axis=mybir.AxisListType.X, op=ALU.add)
        nc.sync.dma_start(cc_in[:], cc_sb[:])
        nc.gpsimd.collective_compute(
            "AllReduce", ALU.add,
            replica_groups=[list(range(n_cores))],
            ins=[cc_in.ap().opt()], outs=[cc_out.ap().opt()])
        st_sb = const.tile([COUT, 2], F32)
        nc.sync.dma_start(st_sb[:], cc_out[:])
        mean_t = const.tile([COUT, 1], F32)
        nc.vector.tensor_scalar_mul(mean_t[:], st_sb[:, 0:1], 1.0 / TOT)
        e2_t = const.tile([COUT, 1], F32)
        nc.vector.tensor_scalar_mul(e2_t[:], st_sb[:, 1:2], 1.0 / TOT)
        msq_t = const.tile([COUT, 1], F32)
        nc.vector.tensor_mul(msq_t[:], mean_t[:], mean_t[:])
        var_t = const.tile([COUT, 1], F32)
        nc.vector.tensor_sub(var_t[:], e2_t[:], msq_t[:])
        std_t = const.tile([COUT, 1], F32)
        nc.scalar.activation(std_t[:], var_t[:], ACTF.Sqrt, bias=eps_t[:])
        rstd_t = const.tile([COUT, 1], F32)
        nc.vector.reciprocal(rstd_t[:], std_t[:])
        scale_t = const.tile([COUT, 1], F32)
        nc.vector.tensor_mul(scale_t[:], gam_t[:], rstd_t[:])
        tmp_t = const.tile([COUT, 1], F32)
        nc.vector.tensor_mul(tmp_t[:], mean_t[:], scale_t[:])
        bias_t = const.tile([COUT, 1], F32)
        nc.vector.tensor_sub(bias_t[:], bet_t[:], tmp_t[:])

        # ---- pass 2: out = relu(scale*prebn + bias), split ACT/DVE ----
        P2 = 2048
        n2 = HW // P2
        for i2 in range(n2):
            i = i2 * P2
            o_t = p2p.tile([COUT, P2], BF16, tag="o_t")
            if i2 % 3 == 0:
                nc.scalar.activation(o_t[:], prebn[:, i:i + P2], ACTF.Relu,
                                     bias=bias_t[:], scale=scale_t[:])
            else:
                nc.vector.tensor_scalar(o_t[:], prebn[:, i:i + P2],
                                        scale_t[:], bias_t[:],
                                        op0=ALU.mult, op1=ALU.add)
                nc.vector.tensor_scalar_max(o_t[:], o_t[:], 0.0)
            eng = nc.sync if i2 % 2 == 0 else nc.scalar
            eng.dma_start(OUT[:, i:i + P2], o_t[:])

    return nc


def make_host_inputs(x_i, mask_i, W, b, gamma, beta, B=8):
    """Per-core in_map for one image shard (host-side constant prep)."""
    nrows = B + 2
    WT1 = np.ascontiguousarray(
        W.transpose(1, 2, 3, 0).reshape(CIN, KS * KS * COUT))
    WT = np.concatenate([WT1, WT1], axis=0).astype(NPBF)
    BP2 = np.zeros((128, COUT), NPBF)
    BP2[0, :] = (b / SLIDE).astype(NPBF)
    BP2[64, :] = (b / SLIDE).astype(NPBF)
    ones2 = np.zeros((128, 2), NPBF)
    ones2[0:64, 0] = 1.0
    ones2[64:128, 1] = 1.0
    T3 = np.zeros((2 * nrows, 2 * B), NPBF)
    for band in range(2):
        for jj in range(B):
            for d in range(3):
                T3[band * nrows + jj + d, band * B + jj] = 1.0
    return {
        "x": np.ascontiguousarray(x_i).astype(NPBF),
        "mask": np.ascontiguousarray(mask_i).astype(NPBF),
        "wt": WT,
        "bp2": BP2,
        "ones2": ones2,
        "t3": T3,
        "gam": gamma.reshape(COUT, 1).astype(np.float32),
        "bet": beta.reshape(COUT, 1).astype(np.float32),
    }


_NC_CACHE = {}


def kernel(x, mask, W, b, gamma, beta):
    x = np.asarray(x)
    mask = np.asarray(mask)
    W = np.asarray(W)
    b = np.asarray(b)
    gamma = np.asarray(gamma)
    beta = np.asarray(beta)
    N, _, H, _ = x.shape
    n_cores = N
    key = (n_cores, H)
    if key not in _NC_CACHE:
        nc = build_nc(n_cores=n_cores, H=H)
        nc.finalize()
        _NC_CACHE[key] = nc
    nc = _NC_CACHE[key]

    in_maps = [make_host_inputs(x[i], mask[i], W, b, gamma, beta)
               for i in range(n_cores)]
    res = run_bass_kernel_spmd(nc, in_maps, core_ids=list(range(n_cores)),
                               trace=bool(os.environ.get("KERNEL_TRACE")))
    out = np.stack([res.results[i]["out"].astype(np.float32)
                    .reshape(COUT, H, W_) for i in range(n_cores)])
    upd = np.stack([res.results[i]["upd"] for i in range(n_cores)])
    update_full = np.broadcast_to(upd[:, None, :, :], (N, COUT, H, W_))
    kernel.last_result = res
    return out, update_full


# revision 6
# speedup vs baseline: 2.2607x; 1.0842x over previous
"""Trainium2 Bass kernel for nn_PartialConvLayer (partial conv 3x3 + mask
update + BatchNorm(batch stats) + ReLU), data-parallel over batch on 8 cores.

Math (per image):
  update = conv(mask, ones(Cin,3,3)), pad 1          # integer in {0..576}
  u      = clip(update, 0, 1)                        # exactly binary
  mr     = 576 / (update + 1e-6) * u
  conv   = conv(x*mask, W), pad 1                    # no bias
  out    = conv * mr + b * u
         = (conv + (b/576) (x) v) * mr,  v = u*(update+1e-6)   [u^2 == u]
  BN over (N,H,W) batch stats (all-reduced across cores), then ReLU.
Returns (out, broadcast(update_clipped)).

v2 design notes (perf):
  - bf16 activations/weights/outputs (tolerance 2e-2; bf16 error ~0.5%).
  - Input loads split into 8-channel-octet DMA instructions across both
    HWDGE queue families so descriptors execute on many rings in parallel
    (v1 crawled at ~57 GB/s on 2 rings, stalling the PE ~590us and keeping
    it clock-gated at 1.2 GHz).
  - Pre-BN activations kept resident in SBUF as bf16 (128 KB/partition),
    killing the 67 MB DRAM bounce of v1's pass 2.
  - band0 (rows 0..63) / band1 (partitions 64..127) conv matmuls issued
    back-to-back so they run concurrently on different PE row groups.
  - v/mru strip relayout via small SBUF->SBUF DMAs instead of DRAM bounce.
"""
import os
import numpy as np
from contextlib import ExitStack

import ml_dtypes

import concourse.bass as bass
import concourse.tile as tile
from concourse import mybir, bacc
from concourse import library_config
from concourse.bass_utils import run_bass_kernel_spmd

F32 = mybir.dt.float32
BF16 = mybir.dt.bfloat16
ALU = mybir.AluOpType
ACTF = mybir.ActivationFunctionType

CIN = 64
COUT = 128
W_ = 256          # image width
KS = 3
EPS_MASK = 1e-6
EPS_BN = 1e-5
SLIDE = float(CIN * KS * KS)   # 576
NPBF = ml_dtypes.bfloat16


def build_nc(n_cores=8, H=256, B=8):
    """SPMD program for one core holding one [CIN, H, W_] image."""
    HB = H // 2                      # rows per band
    nblk = HB // B                   # blocks
    nrows = B + 2                    # rows per band tile (with halo)
    npair = nrows // 2               # row-pairs for the s matmuls
    nchunk = (H * W_) // 512         # 512-px chunks per core
    TOT = float(n_cores * H * W_)    # BN count
    HW = H * W_

    nc = bacc.Bacc(None, num_devices=n_cores)

    X = nc.dram_tensor("x", [CIN, H, W_], BF16, kind="ExternalInput")
    M = nc.dram_tensor("mask", [CIN, H, W_], BF16, kind="ExternalInput")
    WT = nc.dram_tensor("wt", [128, KS * KS * COUT], BF16, kind="ExternalInput")
    BP2 = nc.dram_tensor("bp2", [128, COUT], BF16, kind="ExternalInput")
    ONES2 = nc.dram_tensor("ones2", [128, 2], BF16, kind="ExternalInput")
    T3 = nc.dram_tensor("t3", [2 * nrows, 2 * B], BF16, kind="ExternalInput")
    GAM = nc.dram_tensor("gam", [COUT, 1], F32, kind="ExternalInput")
    BET = nc.dram_tensor("bet", [COUT, 1], F32, kind="ExternalInput")

    OUT = nc.dram_tensor("out", [COUT, HW], BF16, kind="ExternalOutput")
    UPD = nc.dram_tensor("upd", [H, W_], F32, kind="ExternalOutput")

    cc_in = nc.dram_tensor("ccin", [COUT, 2], F32)
    cc_out = nc.dram_tensor("ccout", [COUT, 2], F32,
                            addr_space="Shared" if n_cores > 4 else "Local")

    with tile.TileContext(nc) as tc, ExitStack() as ctx:
        nc.gpsimd.load_library(library_config.mlp)

        const = ctx.enter_context(tc.tile_pool(name="const", bufs=1))
        io = ctx.enter_context(tc.tile_pool(name="io", bufs=2))
        sblk = ctx.enter_context(tc.tile_pool(name="sblk", bufs=2))
        updp = ctx.enter_context(tc.tile_pool(name="updp", bufs=2))
        strp = ctx.enter_context(tc.tile_pool(name="strp", bufs=3))
        sqp = ctx.enter_context(tc.tile_pool(name="sqp", bufs=2))
        p2p = ctx.enter_context(tc.tile_pool(name="p2p", bufs=3))
        psc = ctx.enter_context(tc.tile_pool(name="psc", bufs=2, space="PSUM"))
        pss = ctx.enter_context(tc.tile_pool(name="pss", bufs=2, space="PSUM"))
        psu = ctx.enter_context(tc.tile_pool(name="psu", bufs=2, space="PSUM"))

        # ---- constants ----
        wt_t = const.tile([128, KS * KS * COUT], BF16)
        nc.sync.dma_start(wt_t[:], WT[:])
        bp_t = const.tile([128, COUT], BF16)
        nc.sync.dma_start(bp_t[:], BP2[:])
        ones2_t = const.tile([128, 2], BF16)
        nc.sync.dma_start(ones2_t[:], ONES2[:])
        t3_t = const.tile([2 * nrows, 2 * B], BF16)
        nc.sync.dma_start(t3_t[:], T3[:])
        gam_t = const.tile([COUT, 1], F32)
        nc.sync.dma_start(gam_t[:], GAM[:])
        bet_t = const.tile([COUT, 1], F32)
        nc.sync.dma_start(bet_t[:], BET[:])
        eps_t = const.tile([COUT, 1], F32)
        nc.vector.memset(eps_t[:], EPS_BN)
        sum_slots = const.tile([COUT, nchunk], F32)
        sq_slots = const.tile([COUT, nchunk], F32)
        # pre-BN activations, SBUF-resident for the whole kernel (bf16)
        prebn = const.tile([COUT, HW], BF16)
        # two persistent padded xm buffers; guard cols 0/257 zeroed once
        xm_tiles = []
        for i in range(2):
            t = const.tile([128, nrows * 258], BF16, tag=f"xm{i}")
            nc.vector.memset(t[:], 0.0)
            xm_tiles.append(t)

        def issue_loads(k):
            """Prefetch block k's x/mask band tiles (halo rows), 8-ch octets."""
            r0 = k * B
            x_t = io.tile([128, nrows * W_], BF16, tag="x_t")
            m_t = io.tile([128, nrows * W_], BF16, tag="m_t")
            for tens, tl, eng in ((X, x_t, nc.sync), (M, m_t, nc.scalar)):
                for b in range(2):
                    rlo = b * HB + r0 - 1          # first halo row
                    rhi = rlo + nrows              # one past last halo row
                    lo = max(rlo, 0)
                    hi = min(rhi, H)
                    c0 = (lo - rlo) * W_           # dest col offset
                    nrw = hi - lo
                    for o in range(8):             # channel octets
                        p0 = b * 64 + 8 * o
                        eng.dma_start(
                            tl[p0:p0 + 8, c0:c0 + nrw * W_],
                            bass.AP(tensor=tens, offset=8 * o * HW + lo * W_,
                                    ap=[[HW, 8], [1, nrw * W_]]))
                    if lo > rlo:    # zero top halo row (block 0, band 0)
                        nc.vector.memset(tl[b * 64:b * 64 + 64, 0:W_], 0.0)
                    if hi < rhi:    # zero bottom halo row (last block, band 1)
                        nc.vector.memset(
                            tl[b * 64:b * 64 + 64, (nrows - 1) * W_:nrows * W_], 0.0)
            return x_t, m_t

        ci_global = 0
        pending = [issue_loads(0)]
        for k in range(nblk):
            r0 = k * B
            if k + 1 < nblk:
                pending.append(issue_loads(k + 1))
            x_t, m_t = pending.pop(0)

            x3 = x_t[:, :].rearrange("p (r c) -> p r c", c=W_)
            m3 = m_t[:, :].rearrange("p (r c) -> p r c", c=W_)

            # ---- xm = x*mask into padded bf16 tile ----
            xm = xm_tiles[k % 2]
            xm3 = xm[:, :].rearrange("p (r c) -> p r c", c=258)
            nc.vector.tensor_tensor(xm3[:, :, 1:257], x3, m3, op=ALU.mult)

            # ---- s = cin-sum of mask per row, both bands ----
            s_rows = sblk.tile([2 * nrows, 258], BF16, tag="s_rows")
            nc.vector.memset(s_rows[:, 0:1], 0.0)
            nc.vector.memset(s_rows[:, 257:258], 0.0)
            for p in range(npair):
                ps_s = pss.tile([2, 512], F32, tag="ps_s")
                nc.tensor.matmul(ps_s[:], ones2_t[:], m3[:, 2 * p:2 * p + 2, :],
                                 start=True, stop=True)
                s_pair = sblk.tile([2, 512], BF16, tag="s_pair")
                nc.scalar.copy(s_pair[:], ps_s[:])
                # [2,512] -> rows {2p,2p+1} band0 / {nrows+2p,...} band1
                nc.scalar.dma_start(s_rows[2 * p:2 * p + 2, 1:257],
                                    s_pair[0:1, :])
                nc.scalar.dma_start(
                    s_rows[nrows + 2 * p:nrows + 2 * p + 2, 1:257],
                    s_pair[1:2, :])

            # ---- banded vertical sum via T3 matmul ----
            ps_u = psu.tile([2 * B, 258], F32, tag="ps_u")
            nc.tensor.matmul(ps_u[:], t3_t[:], s_rows[:, :], start=True,
                             stop=True)
            u_sb = updp.tile([2 * B, 258], F32, tag="u_sb")
            nc.scalar.copy(u_sb[:], ps_u[:])

            # ---- horizontal sum + update math  [2B, 256] ----
            vh = updp.tile([2 * B, W_], F32, tag="vh")
            nc.vector.tensor_add(vh[:], u_sb[:, 0:256], u_sb[:, 1:257])
            nc.vector.tensor_add(vh[:], vh[:], u_sb[:, 2:258])
            u_clip = updp.tile([2 * B, W_], F32, tag="u_clip")
            nc.vector.tensor_scalar_min(u_clip[:], vh[:], 1.0)
            upde = updp.tile([2 * B, W_], F32, tag="upde")
            nc.vector.tensor_scalar_add(upde[:], vh[:], EPS_MASK)
            rec = updp.tile([2 * B, W_], F32, tag="rec")
            nc.vector.reciprocal(rec[:], upde[:])
            mru_rows = updp.tile([2 * B, W_], BF16, tag="mru_rows")
            nc.vector.scalar_tensor_tensor(
                out=mru_rows[:], in0=rec[:], scalar=SLIDE, in1=u_clip[:],
                op0=ALU.mult, op1=ALU.mult)
            v_rows = updp.tile([2 * B, W_], BF16, tag="v_rows")
            nc.vector.scalar_tensor_tensor(
                out=v_rows[:], in0=upde[:], scalar=1.0, in1=u_clip[:],
                op0=ALU.mult, op1=ALU.mult)

            nc.scalar.dma_start(
                bass.AP(tensor=UPD, offset=r0 * W_,
                        ap=[[HB * W_, 2], [W_, B], [1, W_]]),
                u_clip[:])

            # ---- conv chunks: per j, band0+band1 paired on row groups ----
            for j in range(0, B, 2):
                vst = strp.tile([128, 512], BF16, tag="vst")
                nc.sync.dma_start(vst[0:1, :], v_rows[j:j + 2, :])
                nc.sync.dma_start(vst[64:65, :], v_rows[B + j:B + j + 2, :])
                mst = strp.tile([1, 1024], BF16, tag="mst")
                nc.sync.dma_start(mst[0:1, 0:512], mru_rows[j:j + 2, :])
                nc.sync.dma_start(mst[0:1, 512:1024],
                                  mru_rows[B + j:B + j + 2, :])
                mru_bc = strp.tile([128, 1024], BF16, tag="mru_bc")
                nc.gpsimd.partition_broadcast(mru_bc[:], mst[0:1, :])

                ps_c0 = psc.tile([COUT, 512], F32, tag="ps_c0")
                ps_c1 = psc.tile([COUT, 512], F32, tag="ps_c1")
                for t in range(KS * KS):
                    ky, kx = divmod(t, KS)
                    lhs_lo = wt_t[0:64, t * COUT:(t + 1) * COUT]
                    lhs_hi = wt_t[64:128, t * COUT:(t + 1) * COUT]
                    nc.tensor.matmul(
                        ps_c0[:], lhs_lo,
                        xm3[0:64, j + ky:j + ky + 2, kx:kx + 256],
                        start=(t == 0), stop=False)
                    nc.tensor.matmul(
                        ps_c1[:], lhs_hi,
                        xm3[64:128, j + ky:j + ky + 2, kx:kx + 256],
                        start=(t == 0), stop=False)
                nc.tensor.matmul(ps_c0[:], bp_t[0:1, :], vst[0:1, :],
                                 start=False, stop=True)
                nc.tensor.matmul(ps_c1[:], bp_t[64:65, :], vst[64:65, :],
                                 start=False, stop=True)

                for b, ps_c in ((0, ps_c0), (1, ps_c1)):
                    off = (b * HB + r0 + j) * W_
                    ci = ci_global + b
                    pslice = prebn[:, off:off + 512]
                    nc.vector.scalar_tensor_tensor(
                        out=pslice, in0=ps_c[:], scalar=0.0,
                        in1=mru_bc[:, 512 * b:512 * b + 512],
                        op0=ALU.add, op1=ALU.mult,
                        accum_out=sum_slots[:, ci:ci + 1])
                    sq_scr = sqp.tile([COUT, 512], BF16, tag="sq_scr")
                    nc.scalar.activation(
                        sq_scr[:], pslice, ACTF.Square,
                        accum_out=sq_slots[:, ci:ci + 1])
                ci_global += 2

        assert ci_global == nchunk

        # ---- BN stats: reduce, all-reduce, affine coeffs ----
        cc_sb = const.tile([COUT, 2], F32)
        nc.vector.tensor_reduce(cc_sb[:, 0:1], sum_slots[:],
                                axis=mybir.AxisListType.X, op=ALU.add)
        nc.vector.tensor_reduce(cc_sb[:, 1:2], sq_slots[:],
                                axis=mybir.AxisListType.X, op=ALU.add)
        nc.sync.dma_start(cc_in[:], cc_sb[:])
        nc.gpsimd.collective_compute(
            "AllReduce", ALU.add,
            replica_groups=[list(range(n_cores))],
            ins=[cc_in.ap().opt()], outs=[cc_out.ap().opt()])
        st_sb = const.tile([COUT, 2], F32)
        nc.sync.dma_start(st_sb[:], cc_out[:])
        mean_t = const.tile([COUT, 1], F32)
        nc.vector.tensor_scalar_mul(mean_t[:], st_sb[:, 0:1], 1.0 / TOT)
        e2_t = const.tile([COUT, 1], F32)
        nc.vector.tensor_scalar_mul(e2_t[:], st_sb[:, 1:2], 1.0 / TOT)
        msq_t = const.tile([COUT, 1], F32)
        nc.vector.tensor_mul(msq_t[:], mean_t[:], mean_t[:])
        var_t = const.tile([COUT, 1], F32)
        nc.vector.tensor_sub(var_t[:], e2_t[:], msq_t[:])
        std_t = const.tile([COUT, 1], F32)
        nc.scalar.activation(std_t[:], var_t[:], ACTF.Sqrt, bias=eps_t[:])
        rstd_t = const.tile([COUT, 1], F32)
        nc.vector.reciprocal(rstd_t[:], std_t[:])
        scale_t = const.tile([COUT, 1], F32)
        nc.vector.tensor_mul(scale_t[:], gam_t[:], rstd_t[:])
        tmp_t = const.tile([COUT, 1], F32)
        nc.vector.tensor_mul(tmp_t[:], mean_t[:], scale_t[:])
        bias_t = const.tile([COUT, 1], F32)
        nc.vector.tensor_sub(bias_t[:], bet_t[:], tmp_t[:])

        # ---- pass 2: out = relu(scale*prebn + bias), split ACT/DVE ----
        P2 = 2048
        n2 = HW // P2
        for i2 in range(n2):
            i = i2 * P2
            o_t = p2p.tile([COUT, P2], BF16, tag="o_t")
            if i2 % 3 == 0:
                nc.scalar.activation(o_t[:], prebn[:, i:i + P2], ACTF.Relu,
                                     bias=bias_t[:], scale=scale_t[:])
            else:
                nc.vector.tensor_scalar(o_t[:], prebn[:, i:i + P2],
                                        scale_t[:], bias_t[:],
                                        op0=ALU.mult, op1=ALU.add)
                nc.vector.tensor_scalar_max(o_t[:], o_t[:], 0.0)
            eng = nc.sync if i2 % 2 == 0 else nc.scalar
            eng.dma_start(OUT[:, i:i + P2], o_t[:])

    return nc


def make_host_inputs(x_i, mask_i, W, b, gamma, beta, B=8):
    """Per-core in_map for one image shard (host-side constant prep)."""
    nrows = B + 2
    WT1 = np.ascontiguousarray(
        W.transpose(1, 2, 3, 0).reshape(CIN, KS * KS * COUT))
    WT = np.concatenate([WT1, WT1], axis=0).astype(NPBF)
    BP2 = np.zeros((128, COUT), NPBF)
    BP2[0, :] = (b / SLIDE).astype(NPBF)
    BP2[64, :] = (b / SLIDE).astype(NPBF)
    ones2 = np.zeros((128, 2), NPBF)
    ones2[0:64, 0] = 1.0
    ones2[64:128, 1] = 1.0
    T3 = np.zeros((2 * nrows, 2 * B), NPBF)
    for band in range(2):
        for jj in range(B):
            for d in range(3):
                T3[band * nrows + jj + d, band * B + jj] = 1.0
    return {
        "x": np.ascontiguousarray(x_i).astype(NPBF),
        "mask": np.ascontiguousarray(mask_i).astype(NPBF),
        "wt": WT,
        "bp2": BP2,
        "ones2": ones2,
        "t3": T3,
        "gam": gamma.reshape(COUT, 1).astype(np.float32),
        "bet": beta.reshape(COUT, 1).astype(np.float32),
    }


_NC_CACHE = {}


def kernel(x, mask, W, b, gamma, beta):
    x = np.asarray(x)
    mask = np.asarray(mask)
    W = np.asarray(W)
    b = np.asarray(b)
    gamma = np.asarray(gamma)
    beta = np.asarray(beta)
    N, _, H, _ = x.shape
    n_cores = N
    key = (n_cores, H)
    if key not in _NC_CACHE:
        nc = build_nc(n_cores=n_cores, H=H)
        nc.finalize()
        _NC_CACHE[key] = nc
    nc = _NC_CACHE[key]

    in_maps = [make_host_inputs(x[i], mask[i], W, b, gamma, beta)
               for i in range(n_cores)]
    res = run_bass_kernel_spmd(nc, in_maps, core_ids=list(range(n_cores)),
                               trace=bool(os.environ.get("KERNEL_TRACE")))
    out = np.stack([res.results[i]["out"].astype(np.float32)
                    .reshape(COUT, H, W_) for i in range(n_cores)])
    upd = np.stack([res.results[i]["upd"] for i in range(n_cores)])
    update_full = np.broadcast_to(upd[:, None, :, :], (N, COUT, H, W_))
    kernel.last_result = res
    return out, update_full


# revision 14
# speedup vs baseline: 3.2467x; 1.4362x over previous
"""Trainium2 Bass kernel for nn_PartialConvLayer (partial conv 3x3 + mask
update + BatchNorm(batch stats) + ReLU), data-parallel over batch on 8 cores.

Math (per image):
  update = conv(mask, ones(Cin,3,3)), pad 1          # integer in {0..576}
  u      = clip(update, 0, 1)                        # exactly binary
  mr     = 576 / (update + 1e-6) * u
  conv   = conv(x*mask, W), pad 1                    # no bias
  out    = conv * mr + b * u
         = (conv + (b/576) (x) v) * mr,  v = u*(update+1e-6)   [u^2 == u]
  BN over (N,H,W) batch stats (all-reduced across cores), then ReLU.
Returns (out, broadcast(update_clipped)).

Perf design:
  - bf16 activations/weights/outputs (tolerance 2e-2; bf16 error ~0.5%).
  - One DMA instruction per tensor per block for input loads, with the
    64-count channel dim outermost so descriptors spray across all DMA
    rings (the DGE sprays on the outermost AP dim). DMA instruction issue
    costs ~600ns of sequencer time, so instruction count is minimized
    everywhere: strips/reshapes are merged via an interleaved row order
    (partition 4*(j//2)+2*band+(j%2)) so each is a single DMA.
  - Pre-BN activations stay resident in SBUF as bf16 (128 KB/partition);
    pass 2 reads SBUF and writes bf16 DRAM.
  - band0 (rows 0..63) / band1 (rows 64..127) conv matmuls issued
    back-to-back so they run concurrently on different PE row groups.
  - Block k+1's loads are issued before block k's compute (prefetch) to
    keep the PE dense and clock-warm.
"""
import os
import numpy as np
from contextlib import ExitStack

import ml_dtypes

import concourse.bass as bass
import concourse.tile as tile
from concourse import mybir, bacc
from concourse import library_config
from concourse.bass_utils import run_bass_kernel_spmd

F32 = mybir.dt.float32
BF16 = mybir.dt.bfloat16
ALU = mybir.AluOpType
ACTF = mybir.ActivationFunctionType

CIN = 64
COUT = 128
W_ = 256          # image width
KS = 3
EPS_MASK = 1e-6
EPS_BN = 1e-5
SLIDE = float(CIN * KS * KS)   # 576
NPBF = ml_dtypes.bfloat16


def build_nc(n_cores=8, H=256, B=8):
    """SPMD program for one core holding one [CIN, H, W_] image."""
    HB = H // 2                      # rows per band
    nblk = HB // B                   # blocks
    nrows = B + 2                    # rows per band tile (with halo)
    npair = nrows // 2               # row-pairs for the s matmuls
    nchunk = (H * W_) // 512         # 512-px chunks per core
    TOT = float(n_cores * H * W_)    # BN count
    HW = H * W_
    NJ = B // 2                      # j-pairs (chunk pairs) per block

    nc = bacc.Bacc(None, num_devices=n_cores)

    # x/mask pre-split into bands on host: partition-row b*64+c holds rows
    # (b*HB-1 .. b*HB+HB) of channel c, zero-padded outside the image.
    X = nc.dram_tensor("x", [128, (HB + 2) * W_], BF16, kind="ExternalInput")
    M = nc.dram_tensor("mask", [128, (HB + 2) * W_], BF16, kind="ExternalInput")
    WT = nc.dram_tensor("wt", [128, KS * KS * COUT], BF16, kind="ExternalInput")
    BP2 = nc.dram_tensor("bp2", [128, COUT], BF16, kind="ExternalInput")
    ONES2 = nc.dram_tensor("ones2", [128, 2], BF16, kind="ExternalInput")
    T3 = nc.dram_tensor("t3", [2 * nrows, 2 * B], BF16, kind="ExternalInput")
    GAM = nc.dram_tensor("gam", [COUT, 1], F32, kind="ExternalInput")
    BET = nc.dram_tensor("bet", [COUT, 1], F32, kind="ExternalInput")

    OUT = nc.dram_tensor("out", [COUT, HW], BF16, kind="ExternalOutput")
    UPD = nc.dram_tensor("upd", [H, W_], F32, kind="ExternalOutput")

    cc_in = nc.dram_tensor("ccin", [COUT, 2], F32)
    cc_out = nc.dram_tensor("ccout", [COUT, 2], F32,
                            addr_space="Shared" if n_cores > 4 else "Local")

    with tile.TileContext(nc) as tc, ExitStack() as ctx:
        nc.gpsimd.load_library(library_config.mlp)

        const = ctx.enter_context(tc.tile_pool(name="const", bufs=1))
        io = ctx.enter_context(tc.tile_pool(name="io", bufs=2))
        sblk = ctx.enter_context(tc.tile_pool(name="sblk", bufs=1))
        updp = ctx.enter_context(tc.tile_pool(name="updp", bufs=2))
        strp = ctx.enter_context(tc.tile_pool(name="strp", bufs=2))
        sqp = ctx.enter_context(tc.tile_pool(name="sqp", bufs=1))
        stp1 = ctx.enter_context(tc.tile_pool(name="stp1", bufs=1))
        p2p = ctx.enter_context(tc.tile_pool(name="p2p", bufs=2))
        psc = ctx.enter_context(tc.tile_pool(name="psc", bufs=2, space="PSUM"))
        pss = ctx.enter_context(tc.tile_pool(name="pss", bufs=2, space="PSUM"))
        psu = ctx.enter_context(tc.tile_pool(name="psu", bufs=2, space="PSUM"))

        # ---- constants ----
        wt_t = const.tile([128, KS * KS * COUT], BF16)
        nc.sync.dma_start(wt_t[:], WT[:])
        bp_t = const.tile([128, COUT], BF16)
        nc.sync.dma_start(bp_t[:], BP2[:])
        ones2_t = const.tile([128, 2], BF16)
        nc.sync.dma_start(ones2_t[:], ONES2[:])
        t3_t = const.tile([2 * nrows, 2 * B], BF16)
        nc.sync.dma_start(t3_t[:], T3[:])
        gam_t = const.tile([COUT, 1], F32)
        nc.sync.dma_start(gam_t[:], GAM[:])
        bet_t = const.tile([COUT, 1], F32)
        nc.sync.dma_start(bet_t[:], BET[:])
        eps_t = const.tile([COUT, 1], F32)
        nc.vector.memset(eps_t[:], EPS_BN)
        sum_slots = const.tile([COUT, nchunk], F32)
        sq_slots = const.tile([COUT, nchunk], F32)
        # pre-BN activations, SBUF-resident for the whole kernel (bf16)
        prebn = const.tile([COUT, HW], BF16)
        # persistent padded xm buffers; guard cols 0/257 zeroed once
        xm_tiles = []
        for i in range(2):
            t = const.tile([128, nrows * 258], BF16, tag=f"xm{i}")
            nc.vector.memset(t[:], 0.0)
            xm_tiles.append(t)
        # persistent s_rows buffers; guard cols 0/257 zeroed once
        sr_tiles = []
        for i in range(2):
            t = const.tile([2 * nrows, 258], BF16, tag=f"sr{i}")
            nc.vector.memset(t[:], 0.0)
            sr_tiles.append(t)

        def issue_loads(k):
            """Prefetch block k's x/mask band tiles (halo rows included)."""
            r0 = k * B
            x_t = io.tile([128, nrows * W_], BF16, tag="x_t")
            m_t = io.tile([128, nrows * W_], BF16, tag="m_t")
            for tens, tl, eng in ((X, x_t, nc.sync), (M, m_t, nc.scalar)):
                eng.dma_start(
                    tl[:, :],
                    bass.AP(tensor=tens, offset=r0 * W_,
                            ap=[[(HB + 2) * W_, 128], [1, nrows * W_]]))
            return x_t, m_t

        ci_global = 0
        pending = [issue_loads(0)]
        for k in range(nblk):
            r0 = k * B
            if k + 1 < nblk:
                pending.append(issue_loads(k + 1))
            x_t, m_t = pending.pop(0)

            x3 = x_t[:, :].rearrange("p (r c) -> p r c", c=W_)
            m3 = m_t[:, :].rearrange("p (r c) -> p r c", c=W_)

            # ---- xm = x*mask into padded bf16 tile ----
            xm = xm_tiles[k % 2]
            xm3 = xm[:, :].rearrange("p (r c) -> p r c", c=258)
            nc.vector.tensor_tensor(xm3[:, :, 1:257], x3, m3, op=ALU.mult)

            # ---- s = cin-sum of mask per row-pair, both bands ----
            s_all = sblk.tile([2, npair * 512], BF16, tag="s_all")
            for p in range(npair):
                ps_s = pss.tile([2, 512], F32, tag="ps_s")
                nc.tensor.matmul(ps_s[:], ones2_t[:], m3[:, 2 * p:2 * p + 2, :],
                                 start=True, stop=True)
                nc.scalar.copy(s_all[:, 512 * p:512 * p + 512], ps_s[:])
            # one reshape DMA: [2, npair*512] -> [2*nrows, 256] rows
            s_rows = sr_tiles[k % 2]
            nc.scalar.dma_start(
                s_rows[:, 1:257],
                s_all[:, :].rearrange("b (r f) -> b r f", f=256))

            # ---- banded vertical sum via T3 matmul ----
            # output row order: partition band*B + j
            ps_u = psu.tile([2 * B, 258], F32, tag="ps_u")
            nc.tensor.matmul(ps_u[:], t3_t[:], s_rows[:, :], start=True,
                             stop=True)
            u_sb = updp.tile([2 * B, 258], F32, tag="u_sb")
            nc.scalar.copy(u_sb[:], ps_u[:])

            # ---- horizontal sum + update math  [2B, 256] ----
            vh = updp.tile([2 * B, W_], F32, tag="vh")
            nc.vector.tensor_add(vh[:], u_sb[:, 0:256], u_sb[:, 1:257])
            nc.vector.tensor_add(vh[:], vh[:], u_sb[:, 2:258])
            u_clip = updp.tile([2 * B, W_], F32, tag="u_clip")
            nc.vector.tensor_scalar_min(u_clip[:], vh[:], 1.0)
            nc.vector.tensor_scalar_add(vh[:], vh[:], EPS_MASK)  # vh -> upde
            rec = updp.tile([2 * B, W_], F32, tag="rec")
            nc.vector.reciprocal(rec[:], vh[:])
            mru_rows = updp.tile([2 * B, W_], BF16, tag="mru_rows")
            nc.vector.scalar_tensor_tensor(
                out=mru_rows[:], in0=rec[:], scalar=SLIDE, in1=u_clip[:],
                op0=ALU.mult, op1=ALU.mult)
            v_rows = updp.tile([2 * B, W_], BF16, tag="v_rows")
            nc.vector.scalar_tensor_tensor(
                out=v_rows[:], in0=vh[:], scalar=1.0, in1=u_clip[:],
                op0=ALU.mult, op1=ALU.mult)

            # UPD out: one DMA (src partitions 0..15 = (band, row) order)
            nc.scalar.dma_start(
                bass.AP(tensor=UPD, offset=r0 * W_,
                        ap=[[HB * W_, 2], [1, B * W_]]),
                u_clip[:, :])

            # strip relayouts: band rows are partition-contiguous
            vst = stp1.tile([128, B * W_], BF16, tag="vst")
            nc.sync.dma_start(vst[0:1, :], v_rows[0:B, :])
            nc.sync.dma_start(vst[64:65, :], v_rows[B:2 * B, :])
            mst0 = stp1.tile([1, B * W_], BF16, tag="mst0")
            nc.sync.dma_start(mst0[:, :], mru_rows[0:B, :])
            mst1 = stp1.tile([1, B * W_], BF16, tag="mst1")
            nc.sync.dma_start(mst1[:, :], mru_rows[B:2 * B, :])

            # ---- conv chunks: per j-pair, band0+band1 on PE row groups ----
            for q in range(NJ):
                j = 2 * q
                mru_bc = strp.tile([128, 1024], BF16, tag="mru_bc")
                nc.gpsimd.partition_broadcast(
                    mru_bc[:, 0:512], mst0[0:1, 512 * q:512 * q + 512])
                nc.gpsimd.partition_broadcast(
                    mru_bc[:, 512:1024], mst1[0:1, 512 * q:512 * q + 512])

                ps_c0 = psc.tile([COUT, 512], F32, tag="ps_c0")
                ps_c1 = psc.tile([COUT, 512], F32, tag="ps_c1")
                for t in range(KS * KS):
                    ky, kx = divmod(t, KS)
                    nc.tensor.matmul(
                        ps_c0[:], wt_t[0:64, t * COUT:(t + 1) * COUT],
                        xm3[0:64, j + ky:j + ky + 2, kx:kx + 256],
                        start=(t == 0), stop=False)
                    nc.tensor.matmul(
                        ps_c1[:], wt_t[64:128, t * COUT:(t + 1) * COUT],
                        xm3[64:128, j + ky:j + ky + 2, kx:kx + 256],
                        start=(t == 0), stop=False)
                nc.tensor.matmul(ps_c0[:], bp_t[0:1, :],
                                 vst[0:1, 512 * q:512 * q + 512],
                                 start=False, stop=True)
                nc.tensor.matmul(ps_c1[:], bp_t[64:65, :],
                                 vst[64:65, 512 * q:512 * q + 512],
                                 start=False, stop=True)

                for b, ps_c in ((0, ps_c0), (1, ps_c1)):
                    off = (b * HB + r0 + j) * W_
                    ci = ci_global + b
                    pslice = prebn[:, off:off + 512]
                    nc.vector.scalar_tensor_tensor(
                        out=pslice, in0=ps_c[:], scalar=0.0,
                        in1=mru_bc[:, 512 * b:512 * b + 512],
                        op0=ALU.add, op1=ALU.mult,
                        accum_out=sum_slots[:, ci:ci + 1])
                    sq_scr = sqp.tile([COUT, 512], BF16, tag="sq_scr")
                    nc.scalar.activation(
                        sq_scr[:], pslice, ACTF.Square,
                        accum_out=sq_slots[:, ci:ci + 1])
                ci_global += 2

        assert ci_global == nchunk

        # ---- BN stats: reduce, all-reduce, affine coeffs ----
        cc_sb = const.tile([COUT, 2], F32)
        nc.vector.tensor_reduce(cc_sb[:, 0:1], sum_slots[:],
                                axis=mybir.AxisListType.X, op=ALU.add)
        nc.vector.tensor_reduce(cc_sb[:, 1:2], sq_slots[:],
                                axis=mybir.AxisListType.X, op=ALU.add)
        nc.sync.dma_start(cc_in[:], cc_sb[:])
        nc.gpsimd.collective_compute(
            "AllReduce", ALU.add,
            replica_groups=[list(range(n_cores))],
            ins=[cc_in.ap().opt()], outs=[cc_out.ap().opt()])
        st_sb = const.tile([COUT, 2], F32)
        nc.sync.dma_start(st_sb[:], cc_out[:])
        mean_t = const.tile([COUT, 1], F32)
        nc.vector.tensor_scalar_mul(mean_t[:], st_sb[:, 0:1], 1.0 / TOT)
        e2_t = const.tile([COUT, 1], F32)
        nc.vector.tensor_scalar_mul(e2_t[:], st_sb[:, 1:2], 1.0 / TOT)
        msq_t = const.tile([COUT, 1], F32)
        nc.vector.tensor_mul(msq_t[:], mean_t[:], mean_t[:])
        var_t = const.tile([COUT, 1], F32)
        nc.vector.tensor_sub(var_t[:], e2_t[:], msq_t[:])
        std_t = const.tile([COUT, 1], F32)
        nc.scalar.activation(std_t[:], var_t[:], ACTF.Sqrt, bias=eps_t[:])
        rstd_t = const.tile([COUT, 1], F32)
        nc.vector.reciprocal(rstd_t[:], std_t[:])
        scale_t = const.tile([COUT, 1], F32)
        nc.vector.tensor_mul(scale_t[:], gam_t[:], rstd_t[:])
        tmp_t = const.tile([COUT, 1], F32)
        nc.vector.tensor_mul(tmp_t[:], mean_t[:], scale_t[:])
        bias_t = const.tile([COUT, 1], F32)
        nc.vector.tensor_sub(bias_t[:], bet_t[:], tmp_t[:])

        # ---- pass 2: out = relu(scale*prebn + bias), split ACT/DVE ----
        P2 = 2048
        n2 = HW // P2
        for i2 in range(n2):
            i = i2 * P2
            o_t = p2p.tile([COUT, P2], BF16, tag="o_t")
            if i2 % 3 == 0:
                nc.scalar.activation(o_t[:], prebn[:, i:i + P2], ACTF.Relu,
                                     bias=bias_t[:], scale=scale_t[:])
            else:
                nc.vector.tensor_scalar(o_t[:], prebn[:, i:i + P2],
                                        scale_t[:], bias_t[:],
                                        op0=ALU.mult, op1=ALU.add)
                nc.vector.tensor_scalar_max(o_t[:], o_t[:], 0.0)
            eng = nc.sync if i2 % 2 == 0 else nc.scalar
            eng.dma_start(OUT[:, i:i + P2], o_t[:])

    return nc


def make_host_inputs(x_i, mask_i, W, b, gamma, beta, B=8):
    """Per-core in_map for one image shard (host-side constant prep)."""
    nrows = B + 2
    WT1 = np.ascontiguousarray(
        W.transpose(1, 2, 3, 0).reshape(CIN, KS * KS * COUT))
    WT = np.concatenate([WT1, WT1], axis=0).astype(NPBF)
    BP2 = np.zeros((128, COUT), NPBF)
    BP2[0, :] = (b / SLIDE).astype(NPBF)
    BP2[64, :] = (b / SLIDE).astype(NPBF)
    ones2 = np.zeros((128, 2), NPBF)
    ones2[0:64, 0] = 1.0
    ones2[64:128, 1] = 1.0
    T3 = np.zeros((2 * nrows, 2 * B), NPBF)
    for band in range(2):
        for jj in range(B):
            for d in range(3):
                T3[band * nrows + jj + d, band * B + jj] = 1.0
    def band_split(a):
        """[CIN, H, W] -> [128, (HB+2)*W]: rows b*HB-1..b*HB+HB, zero-padded."""
        CINL, H, W = a.shape
        HB = H // 2
        ap = np.zeros((CINL, H + 2, W), a.dtype)
        ap[:, 1:H + 1] = a
        out = np.empty((2, CINL, HB + 2, W), a.dtype)
        for b in range(2):
            out[b] = ap[:, b * HB:b * HB + HB + 2]
        return np.ascontiguousarray(
            out.transpose(0, 1, 2, 3).reshape(2 * CINL, (HB + 2) * W))

    return {
        "x": band_split(np.ascontiguousarray(x_i).astype(NPBF)),
        "mask": band_split(np.ascontiguousarray(mask_i).astype(NPBF)),
        "wt": WT,
        "bp2": BP2,
        "ones2": ones2,
        "t3": T3,
        "gam": gamma.reshape(COUT, 1).astype(np.float32),
        "bet": beta.reshape(COUT, 1).astype(np.float32),
    }


_NC_CACHE = {}


def kernel(x, mask, W, b, gamma, beta):
    x = np.asarray(x)
    mask = np.asarray(mask)
    W = np.asarray(W)
    b = np.asarray(b)
    gamma = np.asarray(gamma)
    beta = np.asarray(beta)
    N, _, H, _ = x.shape
    n_cores = N
    key = (n_cores, H)
    if key not in _NC_CACHE:
        nc = build_nc(n_cores=n_cores, H=H)
        nc.finalize()
        _NC_CACHE[key] = nc
    nc = _NC_CACHE[key]

    in_maps = [make_host_inputs(x[i], mask[i], W, b, gamma, beta)
               for i in range(n_cores)]
    res = run_bass_kernel_spmd(nc, in_maps, core_ids=list(range(n_cores)),
                               trace=bool(os.environ.get("KERNEL_TRACE")))
    out = np.stack([res.results[i]["out"].astype(np.float32)
                    .reshape(COUT, H, W_) for i in range(n_cores)])
    upd = np.stack([res.results[i]["upd"] for i in range(n_cores)])
    update_full = np.broadcast_to(upd[:, None, :, :], (N, COUT, H, W_))
    kernel.last_result = res
    return out, update_full


# revision 15
# speedup vs baseline: 3.4152x; 1.0519x over previous
"""Trainium2 Bass kernel for nn_PartialConvLayer (partial conv 3x3 + mask
update + BatchNorm(batch stats) + ReLU), data-parallel over batch on 8 cores.

Math (per image):
  update = conv(mask, ones(Cin,3,3)), pad 1          # integer in {0..576}
  u      = clip(update, 0, 1)                        # exactly binary
  mr     = 576 / (update + 1e-6) * u
  conv   = conv(x*mask, W), pad 1                    # no bias
  out    = conv * mr + b * u
         = (conv + (b/576) (x) v) * mr,  v = u*(update+1e-6)   [u^2 == u]
  BN over (N,H,W) batch stats (all-reduced across cores), then ReLU.
Returns (out, broadcast(update_clipped)).

Perf design:
  - bf16 activations/weights/outputs (tolerance 2e-2; bf16 error ~0.5%).
  - One DMA instruction per tensor per block for input loads, with the
    64-count channel dim outermost so descriptors spray across all DMA
    rings (the DGE sprays on the outermost AP dim). DMA instruction issue
    costs ~600ns of sequencer time, so instruction count is minimized
    everywhere: strips/reshapes are merged via an interleaved row order
    (partition 4*(j//2)+2*band+(j%2)) so each is a single DMA.
  - Pre-BN activations stay resident in SBUF as bf16 (128 KB/partition);
    pass 2 reads SBUF and writes bf16 DRAM.
  - band0 (rows 0..63) / band1 (rows 64..127) conv matmuls issued
    back-to-back so they run concurrently on different PE row groups.
  - Block k+1's loads are issued before block k's compute (prefetch) to
    keep the PE dense and clock-warm.
"""
import os
import numpy as np
from contextlib import ExitStack

import ml_dtypes

import concourse.bass as bass
import concourse.tile as tile
from concourse import mybir, bacc
from concourse import library_config
from concourse.bass_utils import run_bass_kernel_spmd

F32 = mybir.dt.float32
BF16 = mybir.dt.bfloat16
ALU = mybir.AluOpType
ACTF = mybir.ActivationFunctionType

CIN = 64
COUT = 128
W_ = 256          # image width
KS = 3
EPS_MASK = 1e-6
EPS_BN = 1e-5
SLIDE = float(CIN * KS * KS)   # 576
NPBF = ml_dtypes.bfloat16


def build_nc(n_cores=8, H=256, B=8):
    """SPMD program for one core holding one [CIN, H, W_] image."""
    HB = H // 2                      # rows per band
    nblk = HB // B                   # blocks
    nrows = B + 2                    # rows per band tile (with halo)
    npair = nrows // 2               # row-pairs for the s matmuls
    nchunk = (H * W_) // 512         # 512-px chunks per core
    TOT = float(n_cores * H * W_)    # BN count
    HW = H * W_
    NJ = B // 2                      # j-pairs (chunk pairs) per block

    nc = bacc.Bacc(None, num_devices=n_cores)

    # x/mask pre-split into bands on host: partition-row b*64+c holds rows
    # (b*HB-1 .. b*HB+HB) of channel c, zero-padded outside the image.
    X = nc.dram_tensor("x", [128, (HB + 2) * W_], BF16, kind="ExternalInput")
    M = nc.dram_tensor("mask", [128, (HB + 2) * W_], BF16, kind="ExternalInput")
    WT = nc.dram_tensor("wt", [128, KS * KS * COUT], BF16, kind="ExternalInput")
    BP2 = nc.dram_tensor("bp2", [128, COUT], BF16, kind="ExternalInput")
    ONES2 = nc.dram_tensor("ones2", [128, 2], BF16, kind="ExternalInput")
    T3 = nc.dram_tensor("t3", [2 * nrows, 2 * B], BF16, kind="ExternalInput")
    GAM = nc.dram_tensor("gam", [COUT, 1], F32, kind="ExternalInput")
    BET = nc.dram_tensor("bet", [COUT, 1], F32, kind="ExternalInput")

    OUT = nc.dram_tensor("out", [COUT, HW], BF16, kind="ExternalOutput")
    UPD = nc.dram_tensor("upd", [H, W_], F32, kind="ExternalOutput")

    cc_in = nc.dram_tensor("ccin", [COUT, 2], F32)
    cc_out = nc.dram_tensor("ccout", [COUT, 2], F32,
                            addr_space="Shared" if n_cores > 4 else "Local")

    with tile.TileContext(nc) as tc, ExitStack() as ctx:
        nc.gpsimd.load_library(library_config.mlp)

        const = ctx.enter_context(tc.tile_pool(name="const", bufs=1))
        io = ctx.enter_context(tc.tile_pool(name="io", bufs=2))
        sblk = ctx.enter_context(tc.tile_pool(name="sblk", bufs=1))
        updp = ctx.enter_context(tc.tile_pool(name="updp", bufs=2))
        strp = ctx.enter_context(tc.tile_pool(name="strp", bufs=2))
        sqp = ctx.enter_context(tc.tile_pool(name="sqp", bufs=1))
        stp1 = ctx.enter_context(tc.tile_pool(name="stp1", bufs=1))
        p2p = ctx.enter_context(tc.tile_pool(name="p2p", bufs=3))
        psc = ctx.enter_context(tc.tile_pool(name="psc", bufs=2, space="PSUM"))
        pss = ctx.enter_context(tc.tile_pool(name="pss", bufs=2, space="PSUM"))
        psu = ctx.enter_context(tc.tile_pool(name="psu", bufs=2, space="PSUM"))

        # ---- constants ----
        wt_t = const.tile([128, KS * KS * COUT], BF16)
        nc.sync.dma_start(wt_t[:], WT[:])
        bp_t = const.tile([128, COUT], BF16)
        nc.sync.dma_start(bp_t[:], BP2[:])
        ones2_t = const.tile([128, 2], BF16)
        nc.sync.dma_start(ones2_t[:], ONES2[:])
        t3_t = const.tile([2 * nrows, 2 * B], BF16)
        nc.sync.dma_start(t3_t[:], T3[:])
        gam_t = const.tile([COUT, 1], F32)
        nc.sync.dma_start(gam_t[:], GAM[:])
        bet_t = const.tile([COUT, 1], F32)
        nc.sync.dma_start(bet_t[:], BET[:])
        eps_t = const.tile([COUT, 1], F32)
        nc.vector.memset(eps_t[:], EPS_BN)
        sum_slots = const.tile([COUT, nchunk], F32)
        sq_slots = const.tile([COUT, nchunk], F32)
        # pre-BN activations, SBUF-resident for the whole kernel (bf16)
        prebn = const.tile([COUT, HW], BF16)
        # persistent padded xm buffers; guard cols 0/257 zeroed once
        xm_tiles = []
        for i in range(2):
            t = const.tile([128, nrows * 258], BF16, tag=f"xm{i}")
            nc.vector.memset(t[:], 0.0)
            xm_tiles.append(t)
        # persistent s_rows buffers; guard cols 0/257 zeroed once
        sr_tiles = []
        for i in range(2):
            t = const.tile([2 * nrows, 258], BF16, tag=f"sr{i}")
            nc.vector.memset(t[:], 0.0)
            sr_tiles.append(t)

        def issue_loads(k):
            """Prefetch block k's x/mask band tiles (halo rows included)."""
            r0 = k * B
            x_t = io.tile([128, nrows * W_], BF16, tag="x_t")
            m_t = io.tile([128, nrows * W_], BF16, tag="m_t")
            for tens, tl, eng in ((X, x_t, nc.sync), (M, m_t, nc.scalar)):
                eng.dma_start(
                    tl[:, :],
                    bass.AP(tensor=tens, offset=r0 * W_,
                            ap=[[(HB + 2) * W_, 128], [1, nrows * W_]]))
            return x_t, m_t

        ci_global = 0
        pending = [issue_loads(0)]
        for k in range(nblk):
            r0 = k * B
            if k + 1 < nblk:
                pending.append(issue_loads(k + 1))
            x_t, m_t = pending.pop(0)

            x3 = x_t[:, :].rearrange("p (r c) -> p r c", c=W_)
            m3 = m_t[:, :].rearrange("p (r c) -> p r c", c=W_)

            # ---- xm = x*mask into padded bf16 tile ----
            xm = xm_tiles[k % 2]
            xm3 = xm[:, :].rearrange("p (r c) -> p r c", c=258)
            nc.vector.tensor_tensor(xm3[:, :, 1:257], x3, m3, op=ALU.mult)

            # ---- s = cin-sum of mask per row-pair, both bands ----
            s_all = sblk.tile([2, npair * 512], BF16, tag="s_all")
            for p in range(npair):
                ps_s = pss.tile([2, 512], F32, tag="ps_s")
                nc.tensor.matmul(ps_s[:], ones2_t[:], m3[:, 2 * p:2 * p + 2, :],
                                 start=True, stop=True)
                nc.scalar.copy(s_all[:, 512 * p:512 * p + 512], ps_s[:])
            # one reshape DMA: [2, npair*512] -> [2*nrows, 256] rows
            s_rows = sr_tiles[k % 2]
            nc.scalar.dma_start(
                s_rows[:, 1:257],
                s_all[:, :].rearrange("b (r f) -> b r f", f=256))

            # ---- banded vertical sum via T3 matmul ----
            # output row order: partition band*B + j
            ps_u = psu.tile([2 * B, 258], F32, tag="ps_u")
            nc.tensor.matmul(ps_u[:], t3_t[:], s_rows[:, :], start=True,
                             stop=True)
            u_sb = updp.tile([2 * B, 258], F32, tag="u_sb")
            nc.scalar.copy(u_sb[:], ps_u[:])

            # ---- horizontal sum + update math  [2B, 256] ----
            vh = updp.tile([2 * B, W_], F32, tag="vh")
            nc.vector.tensor_add(vh[:], u_sb[:, 0:256], u_sb[:, 1:257])
            nc.vector.tensor_add(vh[:], vh[:], u_sb[:, 2:258])
            u_clip = updp.tile([2 * B, W_], F32, tag="u_clip")
            nc.vector.tensor_scalar_min(u_clip[:], vh[:], 1.0)
            nc.vector.tensor_scalar_add(vh[:], vh[:], EPS_MASK)  # vh -> upde
            rec = updp.tile([2 * B, W_], F32, tag="rec")
            nc.vector.reciprocal(rec[:], vh[:])
            mru_rows = updp.tile([2 * B, W_], BF16, tag="mru_rows")
            nc.vector.scalar_tensor_tensor(
                out=mru_rows[:], in0=rec[:], scalar=SLIDE, in1=u_clip[:],
                op0=ALU.mult, op1=ALU.mult)
            v_rows = updp.tile([2 * B, W_], BF16, tag="v_rows")
            nc.vector.scalar_tensor_tensor(
                out=v_rows[:], in0=vh[:], scalar=1.0, in1=u_clip[:],
                op0=ALU.mult, op1=ALU.mult)

            # UPD out: one DMA (src partitions 0..15 = (band, row) order)
            nc.scalar.dma_start(
                bass.AP(tensor=UPD, offset=r0 * W_,
                        ap=[[HB * W_, 2], [1, B * W_]]),
                u_clip[:, :])

            # mru strip: one DMA, all 16 rows onto partition 0
            mst = stp1.tile([1, 2 * B * W_], BF16, tag="mst")
            nc.sync.dma_start(mst[:, :], mru_rows[:, :])

            # ---- conv chunks: per j-pair, band0+band1 on PE row groups ----
            for q in range(NJ):
                j = 2 * q
                vst = strp.tile([128, 512], BF16, tag="vst")
                nc.sync.dma_start(vst[0:1, :], v_rows[j:j + 2, :])
                nc.scalar.dma_start(vst[64:65, :], v_rows[B + j:B + j + 2, :])
                mru_bc = strp.tile([128, 1024], BF16, tag="mru_bc")
                nc.gpsimd.partition_broadcast(
                    mru_bc[:, 0:512], mst[0:1, 256 * j:256 * j + 512])
                nc.gpsimd.partition_broadcast(
                    mru_bc[:, 512:1024],
                    mst[0:1, 256 * (B + j):256 * (B + j) + 512])

                ps_c0 = psc.tile([COUT, 512], F32, tag="ps_c0")
                ps_c1 = psc.tile([COUT, 512], F32, tag="ps_c1")
                for t in range(KS * KS):
                    ky, kx = divmod(t, KS)
                    nc.tensor.matmul(
                        ps_c0[:], wt_t[0:64, t * COUT:(t + 1) * COUT],
                        xm3[0:64, j + ky:j + ky + 2, kx:kx + 256],
                        start=(t == 0), stop=False)
                    nc.tensor.matmul(
                        ps_c1[:], wt_t[64:128, t * COUT:(t + 1) * COUT],
                        xm3[64:128, j + ky:j + ky + 2, kx:kx + 256],
                        start=(t == 0), stop=False)
                nc.tensor.matmul(ps_c0[:], bp_t[0:1, :], vst[0:1, :],
                                 start=False, stop=True)
                nc.tensor.matmul(ps_c1[:], bp_t[64:65, :], vst[64:65, :],
                                 start=False, stop=True)

                for b, ps_c in ((0, ps_c0), (1, ps_c1)):
                    off = (b * HB + r0 + j) * W_
                    ci = ci_global + b
                    pslice = prebn[:, off:off + 512]
                    nc.vector.scalar_tensor_tensor(
                        out=pslice, in0=ps_c[:], scalar=0.0,
                        in1=mru_bc[:, 512 * b:512 * b + 512],
                        op0=ALU.add, op1=ALU.mult,
                        accum_out=sum_slots[:, ci:ci + 1])
                    sq_scr = sqp.tile([COUT, 512], BF16, tag="sq_scr")
                    nc.scalar.activation(
                        sq_scr[:], pslice, ACTF.Square,
                        accum_out=sq_slots[:, ci:ci + 1])
                ci_global += 2

        assert ci_global == nchunk

        # ---- BN stats: reduce, all-reduce, affine coeffs ----
        cc_sb = const.tile([COUT, 2], F32)
        nc.vector.tensor_reduce(cc_sb[:, 0:1], sum_slots[:],
                                axis=mybir.AxisListType.X, op=ALU.add)
        nc.vector.tensor_reduce(cc_sb[:, 1:2], sq_slots[:],
                                axis=mybir.AxisListType.X, op=ALU.add)
        nc.sync.dma_start(cc_in[:], cc_sb[:])
        nc.gpsimd.collective_compute(
            "AllReduce", ALU.add,
            replica_groups=[list(range(n_cores))],
            ins=[cc_in.ap().opt()], outs=[cc_out.ap().opt()])
        st_sb = const.tile([COUT, 2], F32)
        nc.sync.dma_start(st_sb[:], cc_out[:])
        mean_t = const.tile([COUT, 1], F32)
        nc.vector.tensor_scalar_mul(mean_t[:], st_sb[:, 0:1], 1.0 / TOT)
        e2_t = const.tile([COUT, 1], F32)
        nc.vector.tensor_scalar_mul(e2_t[:], st_sb[:, 1:2], 1.0 / TOT)
        msq_t = const.tile([COUT, 1], F32)
        nc.vector.tensor_mul(msq_t[:], mean_t[:], mean_t[:])
        var_t = const.tile([COUT, 1], F32)
        nc.vector.tensor_sub(var_t[:], e2_t[:], msq_t[:])
        std_t = const.tile([COUT, 1], F32)
        nc.scalar.activation(std_t[:], var_t[:], ACTF.Sqrt, bias=eps_t[:])
        rstd_t = const.tile([COUT, 1], F32)
        nc.vector.reciprocal(rstd_t[:], std_t[:])
        scale_t = const.tile([COUT, 1], F32)
        nc.vector.tensor_mul(scale_t[:], gam_t[:], rstd_t[:])
        tmp_t = const.tile([COUT, 1], F32)
        nc.vector.tensor_mul(tmp_t[:], mean_t[:], scale_t[:])
        bias_t = const.tile([COUT, 1], F32)
        nc.vector.tensor_sub(bias_t[:], bet_t[:], tmp_t[:])

        # ---- pass 2: out = relu(scale*prebn + bias), split ACT/DVE ----
        P2 = 2048
        n2 = HW // P2
        for i2 in range(n2):
            i = i2 * P2
            o_t = p2p.tile([COUT, P2], BF16, tag="o_t")
            if (i2 % 12) in (0, 2, 5, 7, 10):
                nc.scalar.activation(o_t[:], prebn[:, i:i + P2], ACTF.Relu,
                                     bias=bias_t[:], scale=scale_t[:])
            else:
                nc.vector.tensor_scalar(o_t[:], prebn[:, i:i + P2],
                                        scale_t[:], bias_t[:],
                                        op0=ALU.mult, op1=ALU.add)
                nc.vector.tensor_scalar_max(o_t[:], o_t[:], 0.0)
            eng = nc.sync if i2 % 2 == 0 else nc.scalar
            eng.dma_start(OUT[:, i:i + P2], o_t[:])

    return nc


def make_host_inputs(x_i, mask_i, W, b, gamma, beta, B=8):
    """Per-core in_map for one image shard (host-side constant prep)."""
    nrows = B + 2
    WT1 = np.ascontiguousarray(
        W.transpose(1, 2, 3, 0).reshape(CIN, KS * KS * COUT))
    WT = np.concatenate([WT1, WT1], axis=0).astype(NPBF)
    BP2 = np.zeros((128, COUT), NPBF)
    BP2[0, :] = (b / SLIDE).astype(NPBF)
    BP2[64, :] = (b / SLIDE).astype(NPBF)
    ones2 = np.zeros((128, 2), NPBF)
    ones2[0:64, 0] = 1.0
    ones2[64:128, 1] = 1.0
    T3 = np.zeros((2 * nrows, 2 * B), NPBF)
    for band in range(2):
        for jj in range(B):
            for d in range(3):
                T3[band * nrows + jj + d, band * B + jj] = 1.0
    def band_split(a):
        """[CIN, H, W] -> [128, (HB+2)*W]: rows b*HB-1..b*HB+HB, zero-padded."""
        CINL, H, W = a.shape
        HB = H // 2
        ap = np.zeros((CINL, H + 2, W), a.dtype)
        ap[:, 1:H + 1] = a
        out = np.empty((2, CINL, HB + 2, W), a.dtype)
        for b in range(2):
            out[b] = ap[:, b * HB:b * HB + HB + 2]
        return np.ascontiguousarray(
            out.transpose(0, 1, 2, 3).reshape(2 * CINL, (HB + 2) * W))

    return {
        "x": band_split(np.ascontiguousarray(x_i).astype(NPBF)),
        "mask": band_split(np.ascontiguousarray(mask_i).astype(NPBF)),
        "wt": WT,
        "bp2": BP2,
        "ones2": ones2,
        "t3": T3,
        "gam": gamma.reshape(COUT, 1).astype(np.float32),
        "bet": beta.reshape(COUT, 1).astype(np.float32),
    }


_NC_CACHE = {}


def kernel(x, mask, W, b, gamma, beta):
    x = np.asarray(x)
    mask = np.asarray(mask)
    W = np.asarray(W)
    b = np.asarray(b)
    gamma = np.asarray(gamma)
    beta = np.asarray(beta)
    N, _, H, _ = x.shape
    n_cores = N
    key = (n_cores, H)
    if key not in _NC_CACHE:
        nc = build_nc(n_cores=n_cores, H=H)
        nc.finalize()
        _NC_CACHE[key] = nc
    nc = _NC_CACHE[key]

    in_maps = [make_host_inputs(x[i], mask[i], W, b, gamma, beta)
               for i in range(n_cores)]
    res = run_bass_kernel_spmd(nc, in_maps, core_ids=list(range(n_cores)),
                               trace=bool(os.environ.get("KERNEL_TRACE")))
    out = np.stack([res.results[i]["out"].astype(np.float32)
                    .reshape(COUT, H, W_) for i in range(n_cores)])
    upd = np.stack([res.results[i]["upd"] for i in range(n_cores)])
    update_full = np.broadcast_to(upd[:, None, :, :], (N, COUT, H, W_))
    kernel.last_result = res
    return out, update_full
